# revision 1
# baseline (speedup 1.0000x reference)
"""Trainium2 Bass kernel for Points3DLoss (robust chamfer loss).

Computes, for inputs obs (2,16,4096,3) and pred (2,16,2048,3):
  d[bt,n]  = min_m |obs[bt,n] - pred[bt,m]|^2          (chamfer, per frame)
  res      = sqrt(d) reshaped to (B, T*N)
  med, mad = lower-median robust stats per batch row (on detached res)
  w        = bisquare weights; loss = 0.5 * sum(w * res^2)

Strategy: data-parallel over the 32 frames (4 per core). Each core computes
its frames' min-distances via PE matmuls (augmented K=4 dot products:
z = a.b - 0.5|b|^2, so min d = |a|^2 - 2 max z) with the row-max done by
fused DVE tensor_tensor_reduce over PSUM. A single AllGather shares the
per-frame min-distances (d, not res - sqrt is folded into the threshold
algebra); every core then redundantly computes the median/mad via bitwise
bisection on f32 bit patterns (counting passes on DVE + partition-sum via
a ones-matmul on PE) and the final weighted sum. Output read from core 0.
"""

import sys

if '/opt/trn_rl_repo' not in sys.path:
    sys.path.insert(0, '/opt/trn_rl_repo')

import numpy as np

B, T, N_OBS, M_PRED = 2, 16, 4096, 2048
BT = B * T
NCORES = 8
F = BT // NCORES          # frames per core = 4
CH = N_OBS // 128         # obs chunks per frame = 32
COLS = F * CH             # d columns per core = 128
NROW = T * N_OBS          # residuals per batch row = 65536
K_MED = float((NROW - 1) // 2 + 1)   # rank (1-based) of lower median = 32768
TUNE = 4.6851
MADSTD = 0.67449

MED_BITS = list(range(30, 8, -1))    # bisect f32 bit pattern of d, bits 30..9
MAD_BITS = list(range(30, 11, -1))   # bisect f32 bit pattern of t,  bits 30..12

_CACHE = {}


def _build_nc(stage="D", reps=1):
    """stage: A=main loop only, B=+allgather, C=+median, D=full kernel."""
    import concourse.bacc as bacc
    import concourse.tile as tile
    from concourse import mybir
    from contextlib import ExitStack

    A = mybir.AluOpType
    AF = mybir.ActivationFunctionType
    f32 = mybir.dt.float32
    u32 = mybir.dt.uint32
    X = mybir.AxisListType.X

    nc = bacc.Bacc("TRN2", target_bir_lowering=False, debug=False,
                   num_devices=NCORES)

    obs_t = nc.dram_tensor("obs_t", [4, F * N_OBS], f32, kind="ExternalInput").ap()
    pred_t = nc.dram_tensor("pred_t", [4, F * M_PRED], f32, kind="ExternalInput").ap()
    obs_sq = nc.dram_tensor("obs_sq", [128, COLS * 3], f32, kind="ExternalInput").ap()
    out_d = nc.dram_tensor("out", [1, 1], f32, kind="ExternalOutput").ap()
    dbg = None
    if stage in ("A", "B", "C"):
        dbg = nc.dram_tensor("dbg", [128, NCORES * COLS], f32,
                             kind="ExternalOutput").ap()

    def emit(tc, pp):

        OBS = pp.tile([4, F * N_OBS], f32, name="OBS", tag="OBS")
        PRED = pp.tile([4, F * M_PRED], f32, name="PRED", tag="PRED")
        OSQ_SRC = pp.tile([128, COLS * 3], f32, name="OSQ_SRC", tag="OSQ_SRC")
        nc.sync.dma_start(out=OBS, in_=obs_t)
        nc.sync.dma_start(out=PRED, in_=pred_t)
        nc.sync.dma_start(out=OSQ_SRC, in_=obs_sq)

        # |a|^2 per obs point, laid out [p, (f c)] to match zmax columns
        osq = pp.tile([128, COLS * 3], f32, name="osq", tag="osq")
        onorm = pp.tile([128, COLS], f32, name="onorm", tag="onorm")
        nc.scalar.activation(out=osq, in_=OSQ_SRC, func=AF.Square)
        nc.vector.tensor_reduce(
            out=onorm, in_=osq.rearrange("p (c d) -> p c d", d=3), axis=X, op=A.add)

        # lhsT for pred-norm matmul: contract rows 1-3 (coords) with -0.5,
        # ignore row 0 (holds garbage squares of the evolving norm row)
        neg_half = pp.tile([4, 128], f32, name="neg_half", tag="neg_half")
        nc.vector.memset(neg_half, -0.5)
        nc.vector.memset(neg_half[0:1, :], 0.0)
        ones128 = pp.tile([128, 128], f32, name="ones128", tag="ones128")
        nc.vector.memset(ones128, 1.0)
        negones = pp.tile([128, 128], f32, name="negones", tag="negones")
        nc.vector.memset(negones, -1.0)
        half1 = pp.tile([128, 1], f32, name="half1", tag="half1")
        nc.vector.memset(half1, 0.5)

        # --- prep: PRED row 0 = -0.5 * |b|^2 (per frame) ---------------------
        with tc.tile_pool(name="prep_ps", bufs=2, space="PSUM") as auxp, \
             tc.tile_pool(name="prep_sb", bufs=2) as sqp:
            for f in range(F):
                sq = sqp.tile([4, M_PRED], f32, name=f"sq{f}", tag="sq")
                nc.scalar.activation(
                    out=sq, in_=PRED[0:4, f * M_PRED:(f + 1) * M_PRED],
                    func=AF.Square)
                for q in range(4):
                    pn = auxp.tile([128, 512], f32, name=f"pn{f}_{q}", tag="pn")
                    nc.tensor.matmul(pn, lhsT=neg_half,
                                     rhs=sq[:, q * 512:(q + 1) * 512],
                                     start=True, stop=True)
                    lo = f * M_PRED + q * 512
                    nc.scalar.copy(out=PRED[0:1, lo:lo + 512], in_=pn[0:1, :])

        # --- main: z[n,m] = a.b - 0.5|b|^2 ; zmax = max_m z ------------------
        # Row-max split for engine balance: DVE max-reduces PSUM bank 0
        # directly (1x from PSUM), ACT stages banks 1-3 to SBUF where DVE
        # max-reduces at 2x, chaining the partial via the scalar2 init.
        zmax = pp.tile([128, COLS], f32, name="zmax", tag="zmax")
        junk = pp.tile([128, 1536], f32, name="junk", tag="junk")
        ztmp = pp.tile([128, 1], f32, name="ztmp", tag="ztmp")
        with tc.tile_pool(name="mm", bufs=2, space="PSUM") as mmp, \
             tc.tile_pool(name="cpyp", bufs=3) as cpyp:
            for f in range(F):
                for c in range(CH):
                    ps = mmp.tile([128, M_PRED], f32, name="mmps", tag="mmps")
                    lhsT = OBS[:, f * N_OBS + c * 128: f * N_OBS + (c + 1) * 128]
                    for q in range(4):
                        nc.tensor.matmul(
                            ps[:, q * 512:(q + 1) * 512], lhsT=lhsT,
                            rhs=PRED[:, f * M_PRED + q * 512: f * M_PRED + (q + 1) * 512],
                            start=True, stop=True)
                    cpy = cpyp.tile([128, 1536], f32, name="cpy", tag="cpy")
                    nc.scalar.copy(out=cpy, in_=ps[:, 512:2048])
                    nc.vector.tensor_scalar(
                        out=junk[:, 0:512], in0=ps[:, 0:512], scalar1=-1e30,
                        scalar2=None, op0=A.max, op1=A.max, accum_out=ztmp)
                    nc.vector.tensor_scalar(
                        out=junk, in0=cpy, scalar1=-1e30, scalar2=ztmp,
                        op0=A.max, op1=A.max,
                        accum_out=zmax[:, f * CH + c: f * CH + c + 1])

        # d = max(|a|^2 - 2*zmax, 0)
        d_all = pp.tile([128, COLS], f32, name="d_all", tag="d_all")
        nc.vector.scalar_tensor_tensor(
            out=d_all, in0=zmax, scalar=-2.0, op0=A.mult, op1=A.add, in1=onorm)
        nc.vector.tensor_scalar_max(d_all, d_all, 0.0)

        if stage == "A":
            nc.sync.dma_start(out=dbg[:, 0:COLS], in_=d_all)
            return

        # --- allgather d across the 8 cores ---------------------------------
        if stage == "T":
            # timeline-sim variant: no collective (single-core cost model);
            # fake the gather by replicating local d
            g = pp.tile([128, NCORES * COLS], f32, name="g", tag="g")
            for r in range(NCORES):
                nc.scalar.copy(out=g[:, r * COLS:(r + 1) * COLS], in_=d_all)
        else:
            with tc.tile_pool(name="dram", bufs=1, space="DRAM") as dp:
                cc_in = dp.tile([128, COLS], f32, name="cc_in")
                cc_out = dp.tile([NCORES, 128, COLS], f32, name="cc_out",
                                 addr_space="Shared")
                nc.sync.dma_start(out=cc_in, in_=d_all)
                nc.gpsimd.collective_compute(
                    "AllGather", A.bypass,
                    replica_groups=[list(range(NCORES))],
                    ins=[cc_in[:]], outs=[cc_out[:]])
                g = pp.tile([128, NCORES * COLS], f32, name="g", tag="g")
                nc.sync.dma_start(
                    out=g.rearrange("p (r c) -> p r c", r=NCORES),
                    in_=cc_out.rearrange("r p c -> p r c"))

        if stage == "B":
            nc.sync.dma_start(out=dbg, in_=g)
            return

        d0 = g[:, 0:512]      # batch row 0 (cores 0-3)
        d1 = g[:, 512:1024]   # batch row 1 (cores 4-7)

        cnt2 = pp.tile([128, 2], f32, name="cnt2", tag="cnt2")
        delta = pp.tile([128, 2], f32, name="delta", tag="delta")
        jk = junk[:, 0:512]

        with tc.tile_pool(name="bis_ps", bufs=2, space="PSUM") as bp:

            def bisect_med():
                Ts = pp.tile([128, 2], f32, name="Ts_med", tag="Ts_med")
                nc.vector.memset(Ts, float(2 ** 21))
                Tu = pp.tile([128, 2], u32, name="Tu_med", tag="Tu_med")
                nc.vector.tensor_scalar(out=Tu, in0=Ts, scalar1=512.0,
                                        scalar2=None, op0=A.mult)
                Tf = Tu.bitcast(f32)
                for j in MED_BITS:
                    nc.vector.tensor_scalar(
                        out=jk, in0=d0, scalar1=Tf[:, 0:1], scalar2=None,
                        op0=A.is_lt, op1=A.add, accum_out=cnt2[:, 0:1])
                    nc.vector.tensor_scalar(
                        out=jk, in0=d1, scalar1=Tf[:, 1:2], scalar2=None,
                        op0=A.is_lt, op1=A.add, accum_out=cnt2[:, 1:2])
                    tot = bp.tile([128, 2], f32, name="tot_med", tag="tot")
                    nc.tensor.matmul(tot, lhsT=ones128, rhs=cnt2,
                                     start=True, stop=True)
                    nc.vector.tensor_scalar(
                        out=delta, in0=tot, scalar1=K_MED,
                        scalar2=float(2 ** (j - 9)), op0=A.is_lt, op1=A.mult)
                    nc.vector.scalar_tensor_tensor(
                        out=Ts, in0=delta, scalar=float(2 ** (j - 10)),
                        op0=A.subtract, op1=A.add, in1=Ts)
                    nc.vector.tensor_scalar(out=Tu, in0=Ts, scalar1=512.0,
                                            scalar2=None, op0=A.mult)
                return Tf  # center-of-bracket estimate of median(d) per row

            med_d = bisect_med()
            med = pp.tile([128, 2], f32, name="med", tag="med")
            nc.scalar.activation(out=med, in_=med_d, func=AF.Sqrt)

            if stage == "C":
                nc.sync.dma_start(out=dbg[:, 0:2], in_=med_d)
                nc.sync.dma_start(out=dbg[:, 2:4], in_=med)
                return

            def bisect_mad():
                Ts = pp.tile([128, 2], f32, name="Ts_mad", tag="Ts_mad")
                nc.vector.memset(Ts, float(2 ** 21))
                Tu = pp.tile([128, 2], u32, name="Tu_mad", tag="Tu_mad")
                nc.vector.tensor_scalar(out=Tu, in0=Ts, scalar1=512.0,
                                        scalar2=None, op0=A.mult)
                Tf = Tu.bitcast(f32)
                splus = pp.tile([128, 2], f32, name="splus", tag="splus")
                sminus = pp.tile([128, 2], f32, name="sminus", tag="sminus")
                a2 = pp.tile([128, 2], f32, name="a2", tag="a2")
                b2 = pp.tile([128, 2], f32, name="b2", tag="b2")
                cnta = pp.tile([128, 2], f32, name="cnta", tag="cnta")
                cntb = pp.tile([128, 2], f32, name="cntb", tag="cntb")
                for j in MAD_BITS:
                    # band thresholds in d-domain: a=(med+t)^2, b=max(med-t,0)^2
                    nc.vector.tensor_tensor(out=splus, in0=med, in1=Tf, op=A.add)
                    nc.vector.tensor_tensor(out=sminus, in0=med, in1=Tf,
                                            op=A.subtract)
                    nc.vector.tensor_scalar_max(sminus, sminus, 0.0)
                    nc.vector.tensor_tensor(out=a2, in0=splus, in1=splus,
                                            op=A.mult)
                    nc.vector.tensor_tensor(out=b2, in0=sminus, in1=sminus,
                                            op=A.mult)
                    for r, dr in ((0, d0), (1, d1)):
                        nc.vector.tensor_scalar(
                            out=jk, in0=dr, scalar1=a2[:, r:r + 1],
                            scalar2=None, op0=A.is_le, op1=A.add,
                            accum_out=cnta[:, r:r + 1])
                        nc.vector.tensor_scalar(
                            out=jk, in0=dr, scalar1=b2[:, r:r + 1],
                            scalar2=None, op0=A.is_lt, op1=A.add,
                            accum_out=cntb[:, r:r + 1])
                    tot = bp.tile([128, 2], f32, name="tot_mad", tag="tot")
                    nc.tensor.matmul(tot, lhsT=ones128, rhs=cnta,
                                     start=True, stop=False)
                    nc.tensor.matmul(tot, lhsT=negones, rhs=cntb,
                                     start=False, stop=True)
                    nc.vector.tensor_scalar(
                        out=delta, in0=tot, scalar1=K_MED,
                        scalar2=float(2 ** (j - 9)), op0=A.is_lt, op1=A.mult)
                    nc.vector.scalar_tensor_tensor(
                        out=Ts, in0=delta, scalar=float(2 ** (j - 10)),
                        op0=A.subtract, op1=A.add, in1=Ts)
                    nc.vector.tensor_scalar(out=Tu, in0=Ts, scalar1=512.0,
                                            scalar2=None, op0=A.mult)
                return Tf  # mad estimate (res domain) per row

            mad = bisect_mad()

            # --- loss = 0.5 * sum over rows of sum(w * d),
            #     w = relu(1 - d/(TUNE*std)^2)^2, std = mad/MADSTD ------------
            c1 = pp.tile([128, 2], f32, name="c1", tag="c1")
            nc.vector.tensor_scalar(out=c1, in0=mad, scalar1=TUNE / MADSTD,
                                    scalar2=None, op0=A.mult)
            cs2 = pp.tile([128, 2], f32, name="cs2", tag="cs2")
            nc.vector.tensor_tensor(out=cs2, in0=c1, in1=c1, op=A.mult)
            inv = pp.tile([128, 2], f32, name="inv", tag="inv")
            nc.vector.reciprocal(inv, cs2)

            S = pp.tile([128, 2], f32, name="S", tag="S")
            v = pp.tile([128, 512], f32, name="v", tag="v")
            y = pp.tile([128, 512], f32, name="y", tag="y")
            for r, dr in ((0, d0), (1, d1)):
                nc.vector.tensor_scalar(out=jk, in0=dr,
                                        scalar1=inv[:, r:r + 1], scalar2=None,
                                        op0=A.mult)
                nc.scalar.activation(out=v, in_=jk, func=AF.Relu,
                                     bias=1.0, scale=-1.0)
                nc.vector.tensor_tensor(out=y, in0=v, in1=dr, op=A.mult)
                nc.vector.scalar_tensor_tensor(
                    out=jk, in0=y, scalar=1.0, op0=A.bypass, op1=A.mult,
                    in1=v, accum_out=S[:, r:r + 1])

            ls = bp.tile([1, 2], f32, name="ls")
            nc.tensor.matmul(ls, lhsT=half1, rhs=S, start=True, stop=True)
            ls_sb = pp.tile([1, 2], f32, name="ls_sb", tag="ls_sb")
            nc.scalar.copy(out=ls_sb, in_=ls)
            lt = pp.tile([1, 1], f32, name="lt", tag="lt")
            nc.vector.tensor_tensor(out=lt, in0=ls_sb[0:1, 0:1],
                                    in1=ls_sb[0:1, 1:2], op=A.add)
            nc.sync.dma_start(out=out_d, in_=lt)

    from contextlib import ExitStack
    with tile.TileContext(nc) as tc, ExitStack() as stack:
        pp = stack.enter_context(tc.tile_pool(name="persist", bufs=1))
        for _rep in range(reps):
            emit(tc, pp)

    nc.compile()
    return nc


def _shard_inputs(points3d_obs, points3d_pred):
    obs = np.asarray(points3d_obs, dtype=np.float32).reshape(BT, N_OBS, 3)
    pred = np.asarray(points3d_pred, dtype=np.float32).reshape(BT, M_PRED, 3)
    in_maps = []
    for core in range(NCORES):
        so = obs[core * F:(core + 1) * F]       # [F, N, 3]
        sp = pred[core * F:(core + 1) * F]      # [F, M, 3]
        obs_t = np.concatenate(
            [np.ones((1, F * N_OBS), np.float32),
             so.transpose(2, 0, 1).reshape(3, F * N_OBS)], axis=0)
        pred_t = np.concatenate(
            [np.zeros((1, F * M_PRED), np.float32),
             sp.transpose(2, 0, 1).reshape(3, F * M_PRED)], axis=0)
        obs_sq = np.ascontiguousarray(
            so.reshape(F, CH, 128, 3).transpose(2, 0, 1, 3).reshape(128, COLS * 3))
        in_maps.append({
            "obs_t": np.ascontiguousarray(obs_t),
            "pred_t": np.ascontiguousarray(pred_t),
            "obs_sq": obs_sq,
        })
    return in_maps


def _get_nc(stage="D", reps=1):
    key = f"nc_{stage}_{reps}"
    if key not in _CACHE:
        _CACHE[key] = _build_nc(stage, reps)
    return _CACHE[key]


def run(points3d_obs, points3d_pred, stage="D", **kwargs):
    """Run on hardware; kwargs forwarded to run_bass_kernel_spmd (e.g. trace)."""
    from concourse.bass_utils import run_bass_kernel_spmd
    nc = _get_nc(stage)
    in_maps = _shard_inputs(points3d_obs, points3d_pred)
    res = run_bass_kernel_spmd(nc, in_maps, list(range(NCORES)), **kwargs)
    return res


def kernel(points3d_obs, points3d_pred):
    res = run(points3d_obs, points3d_pred)
    loss = np.float32(res.results[0]["out"][0, 0])
    return np.asarray(loss, dtype=np.float32).reshape(())



# revision 5
# speedup vs baseline: 2.8049x; 2.8049x over previous
"""Trainium2 Bass kernel for Points3DLoss (robust chamfer loss).

Computes, for inputs obs (2,16,4096,3) and pred (2,16,2048,3):
  d[bt,n]  = min_m |obs[bt,n] - pred[bt,m]|^2          (chamfer, per frame)
  res      = sqrt(d) reshaped to (B, T*N)
  med, mad = lower-median robust stats per batch row (on detached res)
  w        = bisquare weights; loss = 0.5 * sum(w * res^2)

Strategy (v2):
- Data-parallel over the 32 frames (4 per core).
- PE matmuls in fp16 split precision (hi/lo) with K=13 contraction rows
  computing z' = -0.5*|a-b|^2 directly in PSUM (no |a|^2-2ab cancellation):
  rows pair (hi/lo of -0.5|a|^2) x ones, ha x hb, la x hb, ha x lb, and
  ones x (hi/lo of -0.5|b|^2). fp16 streams 1 col/cycle vs fp32's 4.
- Row max over the 2048 pred cols split between DVE (direct PSUM
  tensor_reduce, X1 cols) and ACT-staged fp16 in SBUF reduced by a DVE
  tensor_scalar cache-reduce (X2 cols), so both engines drain in parallel
  with the PE.
- d gathered per batch row with subgroup AllGathers ({0-3},{4-7}), first
  half issued mid-loop to overlap the collective with compute.
- Tail (per core, on its own batch row): med/mad via value-space bisection
  (18 iters each) counting on fp16 residuals, bisquare weighted sum, row
  loss DMA'd out; host adds core0 + core4 rows.
"""

import sys

if '/opt/trn_rl_repo' not in sys.path:
    sys.path.insert(0, '/opt/trn_rl_repo')

import numpy as np

B, T, N_OBS, M_PRED = 2, 16, 4096, 2048
BT = B * T
NCORES = 8
F = BT // NCORES          # frames per core = 4
CH = N_OBS // 128         # obs chunks per frame = 32
COLS = F * CH             # d columns per core = 128
NROW = T * N_OBS          # residuals per batch row = 65536
K_MED = 32768.0           # rank (1-based) of lower median
TUNE = 4.6851
MADSTD = 0.67449

X1 = 448                  # PSUM-direct max cols per chunk (DVE tensor_reduce)
X2 = M_PRED - X1          # ACT-staged fp16 cols per chunk (DVE cache-reduce)
N_ITERS = 18              # bisection iterations (bracket [0,16])
GROUPS = [[0, 1, 2, 3], [4, 5, 6, 7]]

_CACHE = {}


def _build_nc(stage="D"):
    """stage: A=d_all only, B=gathered g, D=full kernel."""
    import concourse.bacc as bacc
    import concourse.tile as tile
    from concourse import mybir
    from contextlib import ExitStack

    A = mybir.AluOpType
    AF = mybir.ActivationFunctionType
    f32 = mybir.dt.float32
    f16 = mybir.dt.float16
    X = mybir.AxisListType.X

    nc = bacc.Bacc("TRN2", target_bir_lowering=False, debug=False,
                   num_devices=NCORES)

    obs_in = nc.dram_tensor("obs_in", [13, F * N_OBS], f16,
                            kind="ExternalInput").ap()
    pred_in = nc.dram_tensor("pred_in", [13, F * M_PRED], f16,
                             kind="ExternalInput").ap()
    out_d = nc.dram_tensor("out", [1, 1], f32, kind="ExternalOutput").ap()
    dbg = None
    if stage in ("A", "B"):
        dbg = nc.dram_tensor("dbg", [128, 512], f32,
                             kind="ExternalOutput").ap()

    def emit(tc, pp, stack):
        OBSL = pp.tile([13, F * N_OBS], f16, name="OBSL", tag="OBSL")
        PREDL = pp.tile([13, F * M_PRED], f16, name="PREDL", tag="PREDL")
        nc.sync.dma_start(out=OBSL, in_=obs_in)
        nc.sync.dma_start(out=PREDL, in_=pred_in)

        zP = pp.tile([128, COLS], f32, name="zP", tag="zP")
        zD = pp.tile([128, COLS], f32, name="zD", tag="zD")
        junk16 = pp.tile([128, X2], f16, name="junk16", tag="junk16")
        g = pp.tile([128, 512], f32, name="g", tag="g")

        dp = stack.enter_context(tc.tile_pool(name="dram", bufs=1,
                                              space="DRAM"))
        cc_in = []
        cc_out = []
        for h in range(2):
            cc_in.append(dp.tile([128, 64], f32, name=f"cc_in{h}"))
            cc_out.append(dp.tile([4, 128, 64], f32, name=f"cc_out{h}"))

        def gather_half(h):
            # finalize d for frames [2h, 2h+1] and allgather within the
            # 4-core row group
            dh = pp.tile([128, 64], f32, name=f"dh{h}", tag=f"dh{h}")
            lo = h * 64
            nc.vector.tensor_tensor(out=dh, in0=zP[:, lo:lo + 64],
                                    in1=zD[:, lo:lo + 64], op=A.max)
            nc.vector.tensor_scalar(out=dh, in0=dh, scalar1=-2.0,
                                    scalar2=0.0, op0=A.mult, op1=A.max)
            nc.sync.dma_start(out=cc_in[h], in_=dh)
            nc.gpsimd.collective_compute(
                "AllGather", A.bypass, replica_groups=GROUPS,
                ins=[cc_in[h][:]], outs=[cc_out[h][:]])
            nc.sync.dma_start(
                out=g[:, h * 256:(h + 1) * 256].rearrange(
                    "p (r c) -> p r c", r=4),
                in_=cc_out[h].rearrange("r p c -> p r c"))

        # --- main loop: z' = -0.5*|a-b|^2 via K=13 fp16 matmul ------------
        with tc.tile_pool(name="mm", bufs=2, space="PSUM") as mmp, \
             tc.tile_pool(name="stg", bufs=3) as stgp:
            for f in range(F):
                for c in range(CH):
                    col = f * CH + c
                    ps = mmp.tile([128, M_PRED], f32, name="mmps", tag="mmps")
                    lhsT = OBSL[:, f * N_OBS + c * 128:
                                f * N_OBS + (c + 1) * 128]
                    for q in range(4):
                        nc.tensor.matmul(
                            ps[:, q * 512:(q + 1) * 512], lhsT=lhsT,
                            rhs=PREDL[:, f * M_PRED + q * 512:
                                      f * M_PRED + (q + 1) * 512],
                            start=True, stop=True)
                    staged = stgp.tile([128, X2], f16, name="stg", tag="stg")
                    nc.scalar.copy(out=staged, in_=ps[:, X1:M_PRED])
                    nc.vector.tensor_reduce(
                        out=zP[:, col:col + 1], in_=ps[:, 0:X1], axis=X,
                        op=A.max)
                    nc.vector.tensor_scalar(
                        out=junk16, in0=staged, scalar1=-1e30, scalar2=None,
                        op0=A.max, op1=A.max,
                        accum_out=zD[:, col:col + 1])
                if f == 1:
                    gather_half(0)
            gather_half(1)

        if stage == "A":
            nc.sync.dma_start(out=dbg[:, 0:64], in_=pp.tile(
                [128, 64], f32, name="dh0", tag="dh0"))
            nc.sync.dma_start(out=dbg[:, 64:128], in_=pp.tile(
                [128, 64], f32, name="dh1", tag="dh1"))
            return
        if stage == "B":
            nc.sync.dma_start(out=dbg, in_=g)
            return

        # --- tail: med/mad via value bisection on fp16 residuals ----------
        r16 = pp.tile([128, 512], f16, name="r16", tag="r16")
        nc.scalar.activation(out=r16, in_=g, func=AF.Sqrt)

        ones128 = pp.tile([128, 128], f16, name="ones128", tag="ones128")
        nc.vector.memset(ones128, 1.0)
        half1 = pp.tile([128, 1], f32, name="half1", tag="half1")
        nc.vector.memset(half1, 0.5)

        cnt = pp.tile([128, 1], f16, name="cnt", tag="cnt")
        dT = pp.tile([128, 1], f32, name="dT", tag="dT")
        jk512 = junk16[:, 0:512]

        bp = stack.enter_context(tc.tile_pool(name="bis_ps", bufs=2,
                                              space="PSUM"))

        def bisect(vals, tag):
            Tt = pp.tile([128, 1], f32, name=f"T_{tag}", tag=f"T_{tag}")
            nc.vector.memset(Tt, 8.0)
            for j in range(N_ITERS):
                step = float(8.0 / 2 ** (j + 1))
                nc.vector.tensor_scalar(
                    out=jk512, in0=vals, scalar1=Tt[:, 0:1], scalar2=None,
                    op0=A.is_lt, op1=A.add, accum_out=cnt)
                tot = bp.tile([128, 1], f32, name=f"tot_{tag}", tag="tot")
                nc.tensor.matmul(tot, lhsT=ones128, rhs=cnt,
                                 start=True, stop=True)
                nc.vector.tensor_scalar(
                    out=dT, in0=tot, scalar1=K_MED, scalar2=2.0 * step,
                    op0=A.is_lt, op1=A.mult)
                nc.vector.scalar_tensor_tensor(
                    out=Tt, in0=dT, scalar=step, op0=A.subtract, op1=A.add,
                    in1=Tt)
            return Tt

        med = bisect(r16, "med")
        negmed = pp.tile([128, 1], f32, name="negmed", tag="negmed")
        nc.vector.tensor_scalar(out=negmed, in0=med, scalar1=-1.0,
                                scalar2=None, op0=A.mult)
        u16 = pp.tile([128, 512], f16, name="u16", tag="u16")
        nc.scalar.activation(out=u16, in_=r16, func=AF.Abs,
                             bias=negmed[:, 0:1], scale=1.0)
        mad = bisect(u16, "mad")

        # --- loss = 0.5 * sum(w * d), w = relu(1 - d/(TUNE*std)^2)^2 ------
        c1 = pp.tile([128, 1], f32, name="c1", tag="c1")
        nc.vector.tensor_scalar(out=c1, in0=mad, scalar1=TUNE / MADSTD,
                                scalar2=None, op0=A.mult)
        cs2 = pp.tile([128, 1], f32, name="cs2", tag="cs2")
        nc.vector.tensor_tensor(out=cs2, in0=c1, in1=c1, op=A.mult)
        inv = pp.tile([128, 1], f32, name="inv", tag="inv")
        nc.vector.reciprocal(inv, cs2)

        t1 = pp.tile([128, 512], f32, name="t1", tag="t1")
        nc.vector.tensor_scalar(out=t1, in0=g, scalar1=inv[:, 0:1],
                                scalar2=None, op0=A.mult)
        v = pp.tile([128, 512], f32, name="v", tag="v")
        nc.scalar.activation(out=v, in_=t1, func=AF.Relu,
                             bias=1.0, scale=-1.0)
        y = pp.tile([128, 512], f32, name="y", tag="y")
        nc.vector.tensor_tensor(out=y, in0=v, in1=g, op=A.mult)
        S = pp.tile([128, 1], f32, name="S", tag="S")
        jkf = pp.tile([128, 512], f32, name="jkf", tag="jkf")
        nc.vector.scalar_tensor_tensor(
            out=jkf, in0=y, scalar=1.0, op0=A.bypass, op1=A.mult,
            in1=v, accum_out=S)

        ls = bp.tile([1, 1], f32, name="ls")
        nc.tensor.matmul(ls, lhsT=half1, rhs=S, start=True, stop=True)
        ls_sb = pp.tile([1, 1], f32, name="ls_sb", tag="ls_sb")
        nc.scalar.copy(out=ls_sb, in_=ls)
        nc.sync.dma_start(out=out_d, in_=ls_sb)

    from contextlib import ExitStack
    with tile.TileContext(nc) as tc, ExitStack() as stack:
        pp = stack.enter_context(tc.tile_pool(name="persist", bufs=1))
        emit(tc, pp, stack)

    nc.compile()
    return nc


def _split16(x64):
    hi = x64.astype(np.float16)
    lo = (x64 - hi.astype(np.float64)).astype(np.float16)
    return hi, lo


def _shard_inputs(points3d_obs, points3d_pred):
    obs = np.asarray(points3d_obs, dtype=np.float32).reshape(BT, N_OBS, 3)
    pred = np.asarray(points3d_pred, dtype=np.float32).reshape(BT, M_PRED, 3)
    in_maps = []
    for core in range(NCORES):
        so = obs[core * F:(core + 1) * F]       # [F, N, 3]
        sp = pred[core * F:(core + 1) * F]      # [F, M, 3]

        ha, la = _split16(so.astype(np.float64))
        hna, lna = _split16(-0.5 * (so.astype(np.float64) ** 2).sum(-1))
        hb, lb = _split16(sp.astype(np.float64))
        hnb, lnb = _split16(-0.5 * (sp.astype(np.float64) ** 2).sum(-1))

        onesN = np.ones((F, N_OBS), np.float16)
        onesM = np.ones((F, M_PRED), np.float16)

        # [13, F*N]: hi/lo(-0.5|a|^2), ha, la, ha, 1, 1
        obs_rows = np.stack([
            hna, lna,
            ha[..., 0], ha[..., 1], ha[..., 2],
            la[..., 0], la[..., 1], la[..., 2],
            ha[..., 0], ha[..., 1], ha[..., 2],
            onesN, onesN,
        ], axis=0).reshape(13, F * N_OBS)
        # [13, F*M]: 1, 1, hb, hb, lb, hi/lo(-0.5|b|^2)
        pred_rows = np.stack([
            onesM, onesM,
            hb[..., 0], hb[..., 1], hb[..., 2],
            hb[..., 0], hb[..., 1], hb[..., 2],
            lb[..., 0], lb[..., 1], lb[..., 2],
            hnb, lnb,
        ], axis=0).reshape(13, F * M_PRED)

        in_maps.append({
            "obs_in": np.ascontiguousarray(obs_rows),
            "pred_in": np.ascontiguousarray(pred_rows),
        })
    return in_maps


def _get_nc(stage="D"):
    key = f"nc_{stage}"
    if key not in _CACHE:
        _CACHE[key] = _build_nc(stage)
    return _CACHE[key]


def run(points3d_obs, points3d_pred, stage="D", **kwargs):
    """Run on hardware; kwargs forwarded to run_bass_kernel_spmd."""
    from concourse.bass_utils import run_bass_kernel_spmd
    nc = _get_nc(stage)
    in_maps = _shard_inputs(points3d_obs, points3d_pred)
    res = run_bass_kernel_spmd(nc, in_maps, list(range(NCORES)), **kwargs)
    return res


def kernel(points3d_obs, points3d_pred):
    res = run(points3d_obs, points3d_pred)
    loss = (np.float32(res.results[0]["out"][0, 0])
            + np.float32(res.results[4]["out"][0, 0]))
    return np.asarray(loss, dtype=np.float32).reshape(())


# revision 14
# speedup vs baseline: 2.9072x; 1.0365x over previous
"""Trainium2 Bass kernel for Points3DLoss (robust chamfer loss).

Computes, for inputs obs (2,16,4096,3) and pred (2,16,2048,3):
  d[bt,n]  = min_m |obs[bt,n] - pred[bt,m]|^2          (chamfer, per frame)
  res      = sqrt(d) reshaped to (B, T*N)
  med, mad = lower-median robust stats per batch row (on detached res)
  w        = bisquare weights; loss = 0.5 * sum(w * res^2)

Strategy (v3):
- Data-parallel over the 32 frames (4 per core).
- PE matmuls in bf16 split precision (hi/lo) with K=13 contraction rows
  computing z' = -0.5*|a-b|^2 directly in PSUM (no |a|^2-2ab cancellation).
  bf16 streams 1 col/cycle vs fp32's 4.
- Row max over the 2048 pred cols drained by three engines in parallel:
  DVE tensor_reduce direct from PSUM (X1 cols), ACT stages the rest to
  fp16 SBUF where GPSIMD max-reduces it (ports disjoint from DVE's).
- d gathered per batch row with subgroup AllGathers ({0-3},{4-7}), first
  half issued mid-loop to overlap the collective with compute.
- Tail (per core, on its own batch row): med/mad via value-space bisection
  counting on fp16 residuals, the count pass split DVE (is_lt cache-reduce)
  || ACT (Sign-activation accumulate); both partials are combined by two
  accumulating PE matmuls. Bisquare weighted sum, row loss DMA'd out; host
  adds core0 + core4 rows.
"""

import sys

if '/opt/trn_rl_repo' not in sys.path:
    sys.path.insert(0, '/opt/trn_rl_repo')

import numpy as np

B, T, N_OBS, M_PRED = 2, 16, 4096, 2048
BT = B * T
NCORES = 8
F = BT // NCORES          # frames per core = 4
CH = N_OBS // 128         # obs chunks per frame = 32
COLS = F * CH             # d columns per core = 128
NROW = T * N_OBS          # residuals per batch row = 65536
K_MED = 32768.0           # rank (1-based) of lower median
TUNE = 4.6851
MADSTD = 0.67449

X1 = 600                  # PSUM-direct max cols per chunk (DVE tensor_reduce)
X2 = M_PRED - X1          # ACT-staged fp16 cols: DVE tensor_tensor max fold
                          # (2 read ports) then cache-reduce of the half
N_ITERS = 18              # bisection iterations (bracket [0,16])
TAIL_ACT = False          # split tail count passes DVE || ACT (Sign trick)
FD1 = 280                 # tail count cols on DVE; rest Sign-counted on ACT
FD2 = 512 - FD1
GROUPS = [[0, 1, 2, 3], [4, 5, 6, 7]]

_CACHE = {}


def _build_nc(stage="D"):
    import concourse.bacc as bacc
    import concourse.tile as tile
    from concourse import mybir
    from contextlib import ExitStack

    A = mybir.AluOpType
    AF = mybir.ActivationFunctionType
    f32 = mybir.dt.float32
    f16 = mybir.dt.float16
    bf16 = mybir.dt.bfloat16
    X = mybir.AxisListType.X

    nc = bacc.Bacc("TRN2", target_bir_lowering=False, debug=False,
                   num_devices=NCORES)

    obs_in = nc.dram_tensor("obs_in", [13, F * N_OBS], bf16,
                            kind="ExternalInput").ap()
    pred_in = nc.dram_tensor("pred_in", [13, F * M_PRED], bf16,
                             kind="ExternalInput").ap()
    out_d = nc.dram_tensor("out", [1, 1], f32, kind="ExternalOutput").ap()

    def emit(tc, pp, stack):
        OBSL = pp.tile([13, F * N_OBS], bf16, name="OBSL", tag="OBSL")
        PREDL = pp.tile([13, F * M_PRED], bf16, name="PREDL", tag="PREDL")
        nc.sync.dma_start(out=OBSL, in_=obs_in)
        nc.sync.dma_start(out=PREDL, in_=pred_in)

        zP = pp.tile([128, COLS], f32, name="zP", tag="zP")
        zG = pp.tile([128, COLS], f32, name="zG", tag="zG")
        junkG = pp.tile([128, X2], f16, name="junkG", tag="junkG")
        g = pp.tile([128, 512], f32, name="g", tag="g")

        dp = stack.enter_context(tc.tile_pool(name="dram", bufs=1,
                                              space="DRAM"))
        cc_in = []
        cc_out = []
        for h in range(2):
            cc_in.append(dp.tile([128, 64], f32, name=f"cc_in{h}"))
            cc_out.append(dp.tile([4, 128, 64], f32, name=f"cc_out{h}"))

        def gather_half(h):
            dh = pp.tile([128, 64], f32, name=f"dh{h}", tag=f"dh{h}")
            lo = h * 64
            nc.vector.tensor_scalar(out=dh, in0=zG[:, lo:lo + 64],
                                    scalar1=-2.0, scalar2=0.0,
                                    op0=A.mult, op1=A.max)
            nc.sync.dma_start(out=cc_in[h], in_=dh)
            nc.gpsimd.collective_compute(
                "AllGather", A.bypass, replica_groups=GROUPS,
                ins=[cc_in[h][:]], outs=[cc_out[h][:]])
            nc.sync.dma_start(
                out=g[:, h * 256:(h + 1) * 256].rearrange(
                    "p (r c) -> p r c", r=4),
                in_=cc_out[h].rearrange("r p c -> p r c"))

        # --- main loop: z' = -0.5*|a-b|^2 via K=13 bf16 matmul ------------
        with tc.tile_pool(name="mm", bufs=2, space="PSUM") as mmp, \
             tc.tile_pool(name="stg", bufs=3) as stgp:
            for f in range(F):
                for c in range(CH):
                    col = f * CH + c
                    ps = mmp.tile([128, M_PRED], f32, name="mmps", tag="mmps")
                    lhsT = OBSL[:, f * N_OBS + c * 128:
                                f * N_OBS + (c + 1) * 128]
                    for q in range(4):
                        nc.tensor.matmul(
                            ps[:, q * 512:(q + 1) * 512], lhsT=lhsT,
                            rhs=PREDL[:, f * M_PRED + q * 512:
                                      f * M_PRED + (q + 1) * 512],
                            start=True, stop=True)
                    staged = stgp.tile([128, X2], f16, name="stg", tag="stg")
                    nc.scalar.copy(out=staged, in_=ps[:, X1:M_PRED])
                    nc.vector.tensor_reduce(
                        out=zP[:, col:col + 1], in_=ps[:, 0:X1], axis=X,
                        op=A.max)
                    half = X2 // 2
                    fold = stgp.tile([128, half], f16, name="fold",
                                     tag="fold")
                    nc.vector.tensor_tensor(
                        out=fold, in0=staged[:, 0:half],
                        in1=staged[:, half:2 * half], op=A.max)
                    nc.vector.tensor_scalar(
                        out=junkG[:, 0:half], in0=fold, scalar1=-1e30,
                        scalar2=zP[:, col:col + 1], op0=A.max, op1=A.max,
                        accum_out=zG[:, col:col + 1])
                if f == 1:
                    gather_half(0)
            gather_half(1)

        # --- tail: med/mad via value bisection on fp16 residuals ----------
        r16 = pp.tile([128, 512], f16, name="r16", tag="r16")
        nc.scalar.activation(out=r16, in_=g, func=AF.Sqrt)

        ones128 = pp.tile([128, 128], f32, name="ones128", tag="ones128")
        nc.vector.memset(ones128, 1.0)
        halfm = pp.tile([128, 128], f32, name="halfm", tag="halfm")
        nc.vector.memset(halfm, 0.5)
        half1 = pp.tile([128, 1], f32, name="half1", tag="half1")
        nc.vector.memset(half1, 0.5)

        cnt = pp.tile([128, 1], f32, name="cnt", tag="cnt")
        acc = pp.tile([128, 1], f32, name="acc", tag="acc")
        dT = pp.tile([128, 1], f32, name="dT", tag="dT")
        jk16 = junkG[:, 0:FD1]
        jkA = pp.tile([128, FD2], f16, name="jkA", tag="jkA")

        bp = stack.enter_context(tc.tile_pool(name="bis_ps", bufs=2,
                                              space="PSUM"))

        # count(vals < T) split: DVE is_lt on cols [0:FD1], ACT Sign on
        # [FD1:512] (sum of sign(T - x) = c_below - c_above); combined by
        # two accumulating matmuls: tot = sum(cnt) + 0.5*sum(acc), compared
        # against K - 64*FD2.
        K_ADJ = K_MED - 64.0 * FD2

        def bisect(vals, tag):
            Tt = pp.tile([128, 1], f32, name=f"T_{tag}", tag=f"T_{tag}")
            nc.vector.memset(Tt, 8.0)
            for j in range(N_ITERS):
                step = float(8.0 / 2 ** (j + 1))
                tot = bp.tile([128, 1], f32, name=f"tot_{tag}", tag="tot")
                if TAIL_ACT:
                    nc.vector.tensor_scalar(
                        out=jk16, in0=vals[:, 0:FD1], scalar1=Tt[:, 0:1],
                        scalar2=None, op0=A.is_lt, op1=A.add, accum_out=cnt)
                    nc.scalar.activation(
                        out=jkA, in_=vals[:, FD1:512], func=AF.Sign,
                        bias=Tt[:, 0:1], scale=-1.0, accum_out=acc)
                    nc.tensor.matmul(tot, lhsT=ones128, rhs=cnt,
                                     start=True, stop=False)
                    nc.tensor.matmul(tot, lhsT=halfm, rhs=acc,
                                     start=False, stop=True)
                    kcmp = K_ADJ
                else:
                    nc.vector.tensor_scalar(
                        out=junkG[:, 0:512], in0=vals, scalar1=Tt[:, 0:1],
                        scalar2=None, op0=A.is_lt, op1=A.add, accum_out=cnt)
                    nc.tensor.matmul(tot, lhsT=ones128, rhs=cnt,
                                     start=True, stop=True)
                    kcmp = K_MED
                nc.vector.tensor_scalar(
                    out=dT, in0=tot, scalar1=kcmp, scalar2=2.0 * step,
                    op0=A.is_lt, op1=A.mult)
                nc.vector.scalar_tensor_tensor(
                    out=Tt, in0=dT, scalar=step, op0=A.subtract, op1=A.add,
                    in1=Tt)
            return Tt

        med = bisect(r16, "med")
        negmed = pp.tile([128, 1], f32, name="negmed", tag="negmed")
        nc.vector.tensor_scalar(out=negmed, in0=med, scalar1=-1.0,
                                scalar2=None, op0=A.mult)
        u16 = pp.tile([128, 512], f16, name="u16", tag="u16")
        nc.scalar.activation(out=u16, in_=r16, func=AF.Abs,
                             bias=negmed[:, 0:1], scale=1.0)
        mad = bisect(u16, "mad")

        # --- loss = 0.5 * sum(w * d), w = relu(1 - d/(TUNE*std)^2)^2 ------
        c1 = pp.tile([128, 1], f32, name="c1", tag="c1")
        nc.vector.tensor_scalar(out=c1, in0=mad, scalar1=TUNE / MADSTD,
                                scalar2=None, op0=A.mult)
        cs2 = pp.tile([128, 1], f32, name="cs2", tag="cs2")
        nc.vector.tensor_tensor(out=cs2, in0=c1, in1=c1, op=A.mult)
        inv = pp.tile([128, 1], f32, name="inv", tag="inv")
        nc.vector.reciprocal(inv, cs2)

        t1 = pp.tile([128, 512], f32, name="t1", tag="t1")
        nc.vector.tensor_scalar(out=t1, in0=g, scalar1=inv[:, 0:1],
                                scalar2=None, op0=A.mult)
        v = pp.tile([128, 512], f32, name="v", tag="v")
        nc.scalar.activation(out=v, in_=t1, func=AF.Relu,
                             bias=1.0, scale=-1.0)
        y = pp.tile([128, 512], f32, name="y", tag="y")
        nc.vector.tensor_tensor(out=y, in0=v, in1=g, op=A.mult)
        S = pp.tile([128, 1], f32, name="S", tag="S")
        jkf = pp.tile([128, 512], f32, name="jkf", tag="jkf")
        nc.vector.scalar_tensor_tensor(
            out=jkf, in0=y, scalar=1.0, op0=A.bypass, op1=A.mult,
            in1=v, accum_out=S)

        ls = bp.tile([1, 1], f32, name="ls")
        nc.tensor.matmul(ls, lhsT=half1, rhs=S, start=True, stop=True)
        ls_sb = pp.tile([1, 1], f32, name="ls_sb", tag="ls_sb")
        nc.scalar.copy(out=ls_sb, in_=ls)
        nc.sync.dma_start(out=out_d, in_=ls_sb)

    from contextlib import ExitStack
    with tile.TileContext(nc) as tc, ExitStack() as stack:
        pp = stack.enter_context(tc.tile_pool(name="persist", bufs=1))
        emit(tc, pp, stack)

    nc.compile()
    return nc


def _split16(x64, dt):
    hi = x64.astype(dt)
    lo = (x64 - hi.astype(np.float64)).astype(dt)
    return hi, lo


def _shard_inputs(points3d_obs, points3d_pred):
    import ml_dtypes
    bf16 = ml_dtypes.bfloat16
    obs = np.asarray(points3d_obs, dtype=np.float32).reshape(BT, N_OBS, 3)
    pred = np.asarray(points3d_pred, dtype=np.float32).reshape(BT, M_PRED, 3)
    in_maps = []
    for core in range(NCORES):
        so = obs[core * F:(core + 1) * F]       # [F, N, 3]
        sp = pred[core * F:(core + 1) * F]      # [F, M, 3]

        ha, la = _split16(so.astype(np.float64), bf16)
        hna, lna = _split16(-0.5 * (so.astype(np.float64) ** 2).sum(-1), bf16)
        hb, lb = _split16(sp.astype(np.float64), bf16)
        hnb, lnb = _split16(-0.5 * (sp.astype(np.float64) ** 2).sum(-1), bf16)

        onesN = np.ones((F, N_OBS), bf16)
        onesM = np.ones((F, M_PRED), bf16)

        # [13, F*N]: hi/lo(-0.5|a|^2), ha, la, ha, 1, 1
        obs_rows = np.stack([
            hna, lna,
            ha[..., 0], ha[..., 1], ha[..., 2],
            la[..., 0], la[..., 1], la[..., 2],
            ha[..., 0], ha[..., 1], ha[..., 2],
            onesN, onesN,
        ], axis=0).reshape(13, F * N_OBS)
        # [13, F*M]: 1, 1, hb, hb, lb, hi/lo(-0.5|b|^2)
        pred_rows = np.stack([
            onesM, onesM,
            hb[..., 0], hb[..., 1], hb[..., 2],
            hb[..., 0], hb[..., 1], hb[..., 2],
            lb[..., 0], lb[..., 1], lb[..., 2],
            hnb, lnb,
        ], axis=0).reshape(13, F * M_PRED)

        in_maps.append({
            "obs_in": np.ascontiguousarray(obs_rows),
            "pred_in": np.ascontiguousarray(pred_rows),
        })
    return in_maps


def _get_nc(stage="D"):
    key = f"nc_{stage}"
    if key not in _CACHE:
        _CACHE[key] = _build_nc(stage)
    return _CACHE[key]


def run(points3d_obs, points3d_pred, stage="D", **kwargs):
    """Run on hardware; kwargs forwarded to run_bass_kernel_spmd."""
    from concourse.bass_utils import run_bass_kernel_spmd
    nc = _get_nc(stage)
    in_maps = _shard_inputs(points3d_obs, points3d_pred)
    res = run_bass_kernel_spmd(nc, in_maps, list(range(NCORES)), **kwargs)
    return res


def kernel(points3d_obs, points3d_pred):
    res = run(points3d_obs, points3d_pred)
    loss = (np.float32(res.results[0]["out"][0, 0])
            + np.float32(res.results[4]["out"][0, 0]))
    return np.asarray(loss, dtype=np.float32).reshape(())


# revision 15
# speedup vs baseline: 2.9668x; 1.0205x over previous
"""Trainium2 Bass kernel for Points3DLoss (robust chamfer loss).

Computes, for inputs obs (2,16,4096,3) and pred (2,16,2048,3):
  d[bt,n]  = min_m |obs[bt,n] - pred[bt,m]|^2          (chamfer, per frame)
  res      = sqrt(d) reshaped to (B, T*N)
  med, mad = lower-median robust stats per batch row (on detached res)
  w        = bisquare weights; loss = 0.5 * sum(w * res^2)

Strategy (v3):
- Data-parallel over the 32 frames (4 per core).
- PE matmuls in bf16 split precision (hi/lo) with K=13 contraction rows
  computing z' = -0.5*|a-b|^2 directly in PSUM (no |a|^2-2ab cancellation).
  bf16 streams 1 col/cycle vs fp32's 4.
- Row max over the 2048 pred cols drained by three engines in parallel:
  DVE tensor_reduce direct from PSUM (X1 cols), ACT stages the rest to
  fp16 SBUF where GPSIMD max-reduces it (ports disjoint from DVE's).
- d gathered per batch row with subgroup AllGathers ({0-3},{4-7}), first
  half issued mid-loop to overlap the collective with compute.
- Tail (per core, on its own batch row): med/mad via value-space bisection
  counting on fp16 residuals, the count pass split DVE (is_lt cache-reduce)
  || ACT (Sign-activation accumulate); both partials are combined by two
  accumulating PE matmuls. Bisquare weighted sum, row loss DMA'd out; host
  adds core0 + core4 rows.
"""

import sys

if '/opt/trn_rl_repo' not in sys.path:
    sys.path.insert(0, '/opt/trn_rl_repo')

import numpy as np

B, T, N_OBS, M_PRED = 2, 16, 4096, 2048
BT = B * T
NCORES = 8
F = BT // NCORES          # frames per core = 4
CH = N_OBS // 128         # obs chunks per frame = 32
COLS = F * CH             # d columns per core = 128
NROW = T * N_OBS          # residuals per batch row = 65536
K_MED = 32768.0           # rank (1-based) of lower median
TUNE = 4.6851
MADSTD = 0.67449

X1 = 440                  # PSUM-direct max cols per chunk (DVE tensor_reduce)
X2 = M_PRED - X1          # ACT-staged fp16 cols: DVE tensor_tensor max fold
                          # (2 read ports) then cache-reduce of the half
N_ITERS = 16              # bisection iterations (bracket [0,4])
TAIL_ACT = True           # split tail count passes DVE || ACT (Sign trick)
FD1 = 280                 # tail count cols on DVE; rest Sign-counted on ACT
FD2 = 512 - FD1
GROUPS = [[0, 1, 2, 3], [4, 5, 6, 7]]

_CACHE = {}


def _build_nc(stage="D"):
    import concourse.bacc as bacc
    import concourse.tile as tile
    from concourse import mybir
    from contextlib import ExitStack

    A = mybir.AluOpType
    AF = mybir.ActivationFunctionType
    f32 = mybir.dt.float32
    f16 = mybir.dt.float16
    bf16 = mybir.dt.bfloat16
    X = mybir.AxisListType.X

    nc = bacc.Bacc("TRN2", target_bir_lowering=False, debug=False,
                   num_devices=NCORES)

    obs_in = nc.dram_tensor("obs_in", [13, F * N_OBS], bf16,
                            kind="ExternalInput").ap()
    pred_in = nc.dram_tensor("pred_in", [13, F * M_PRED], bf16,
                             kind="ExternalInput").ap()
    out_d = nc.dram_tensor("out", [1, 1], f32, kind="ExternalOutput").ap()

    def emit(tc, pp, stack):
        OBSL = pp.tile([13, F * N_OBS], bf16, name="OBSL", tag="OBSL")
        PREDL = pp.tile([13, F * M_PRED], bf16, name="PREDL", tag="PREDL")
        for f in range(F):
            nc.sync.dma_start(out=PREDL[:, f * M_PRED:(f + 1) * M_PRED],
                              in_=pred_in[:, f * M_PRED:(f + 1) * M_PRED])
            nc.sync.dma_start(out=OBSL[:, f * N_OBS:(f + 1) * N_OBS],
                              in_=obs_in[:, f * N_OBS:(f + 1) * N_OBS])

        zP = pp.tile([128, COLS], f32, name="zP", tag="zP")
        zG = pp.tile([128, COLS], f32, name="zG", tag="zG")
        junkG = pp.tile([128, X2], f16, name="junkG", tag="junkG")
        g = pp.tile([128, 512], f32, name="g", tag="g")

        dp = stack.enter_context(tc.tile_pool(name="dram", bufs=1,
                                              space="DRAM"))
        cc_in = []
        cc_out = []
        for h in range(2):
            cc_in.append(dp.tile([128, 64], f32, name=f"cc_in{h}"))
            cc_out.append(dp.tile([4, 128, 64], f32, name=f"cc_out{h}"))

        def gather_half(h):
            dh = pp.tile([128, 64], f32, name=f"dh{h}", tag=f"dh{h}")
            lo = h * 64
            nc.vector.tensor_scalar(out=dh, in0=zG[:, lo:lo + 64],
                                    scalar1=-2.0, scalar2=0.0,
                                    op0=A.mult, op1=A.max)
            nc.sync.dma_start(out=cc_in[h], in_=dh)
            nc.gpsimd.collective_compute(
                "AllGather", A.bypass, replica_groups=GROUPS,
                ins=[cc_in[h][:]], outs=[cc_out[h][:]])
            nc.sync.dma_start(
                out=g[:, h * 256:(h + 1) * 256].rearrange(
                    "p (r c) -> p r c", r=4),
                in_=cc_out[h].rearrange("r p c -> p r c"))

        # --- main loop: z' = -0.5*|a-b|^2 via K=13 bf16 matmul ------------
        with tc.tile_pool(name="mm", bufs=2, space="PSUM") as mmp, \
             tc.tile_pool(name="stg", bufs=3) as stgp:
            for f in range(F):
                for c in range(CH):
                    col = f * CH + c
                    ps = mmp.tile([128, M_PRED], f32, name="mmps", tag="mmps")
                    lhsT = OBSL[:, f * N_OBS + c * 128:
                                f * N_OBS + (c + 1) * 128]
                    for q in range(4):
                        nc.tensor.matmul(
                            ps[:, q * 512:(q + 1) * 512], lhsT=lhsT,
                            rhs=PREDL[:, f * M_PRED + q * 512:
                                      f * M_PRED + (q + 1) * 512],
                            start=True, stop=True)
                    staged = stgp.tile([128, X2], f16, name="stg", tag="stg")
                    nc.scalar.copy(out=staged, in_=ps[:, X1:M_PRED])
                    nc.vector.tensor_reduce(
                        out=zP[:, col:col + 1], in_=ps[:, 0:X1], axis=X,
                        op=A.max)
                    half = X2 // 2
                    quart = half // 2
                    fold = stgp.tile([128, half], f16, name="fold",
                                     tag="fold")
                    nc.vector.tensor_tensor(
                        out=fold, in0=staged[:, 0:half],
                        in1=staged[:, half:2 * half], op=A.max)
                    fold2 = stgp.tile([128, quart], f16, name="fold2",
                                      tag="fold2")
                    nc.vector.tensor_tensor(
                        out=fold2, in0=fold[:, 0:quart],
                        in1=fold[:, quart:2 * quart], op=A.max)
                    nc.vector.tensor_scalar(
                        out=junkG[:, 0:quart], in0=fold2, scalar1=-1e30,
                        scalar2=zP[:, col:col + 1], op0=A.max, op1=A.max,
                        accum_out=zG[:, col:col + 1])
                if f == 1:
                    gather_half(0)
            gather_half(1)

        # --- tail: med/mad via value bisection on fp16 residuals ----------
        r16 = pp.tile([128, 512], f16, name="r16", tag="r16")
        nc.scalar.activation(out=r16, in_=g, func=AF.Sqrt)

        ones128 = pp.tile([128, 128], f32, name="ones128", tag="ones128")
        nc.vector.memset(ones128, 1.0)
        halfm = pp.tile([128, 128], f32, name="halfm", tag="halfm")
        nc.vector.memset(halfm, 0.5)
        half1 = pp.tile([128, 1], f32, name="half1", tag="half1")
        nc.vector.memset(half1, 0.5)

        cnt = pp.tile([128, 1], f32, name="cnt", tag="cnt")
        acc = pp.tile([128, 1], f32, name="acc", tag="acc")
        dT = pp.tile([128, 1], f32, name="dT", tag="dT")
        jk16 = junkG[:, 0:FD1]
        jkA = pp.tile([128, FD2], f16, name="jkA", tag="jkA")

        bp = stack.enter_context(tc.tile_pool(name="bis_ps", bufs=2,
                                              space="PSUM"))

        # count(vals < T) split: DVE is_lt on cols [0:FD1], ACT Sign on
        # [FD1:512] (sum of sign(T - x) = c_below - c_above); combined by
        # two accumulating matmuls: tot = sum(cnt) + 0.5*sum(acc), compared
        # against K - 64*FD2.
        K_ADJ = K_MED - 64.0 * FD2

        def bisect(vals, tag):
            Tt = pp.tile([128, 1], f32, name=f"T_{tag}", tag=f"T_{tag}")
            nc.vector.memset(Tt, 2.0)
            for j in range(N_ITERS):
                step = float(2.0 / 2 ** (j + 1))
                tot = bp.tile([128, 1], f32, name=f"tot_{tag}", tag="tot")
                if TAIL_ACT:
                    nc.vector.tensor_scalar(
                        out=jk16, in0=vals[:, 0:FD1], scalar1=Tt[:, 0:1],
                        scalar2=None, op0=A.is_lt, op1=A.add, accum_out=cnt)
                    nc.scalar.activation(
                        out=jkA, in_=vals[:, FD1:512], func=AF.Sign,
                        bias=Tt[:, 0:1], scale=-1.0, accum_out=acc)
                    nc.tensor.matmul(tot, lhsT=ones128, rhs=cnt,
                                     start=True, stop=False)
                    nc.tensor.matmul(tot, lhsT=halfm, rhs=acc,
                                     start=False, stop=True)
                    kcmp = K_ADJ
                else:
                    nc.vector.tensor_scalar(
                        out=junkG[:, 0:512], in0=vals, scalar1=Tt[:, 0:1],
                        scalar2=None, op0=A.is_lt, op1=A.add, accum_out=cnt)
                    nc.tensor.matmul(tot, lhsT=ones128, rhs=cnt,
                                     start=True, stop=True)
                    kcmp = K_MED
                nc.vector.tensor_scalar(
                    out=dT, in0=tot, scalar1=kcmp, scalar2=2.0 * step,
                    op0=A.is_lt, op1=A.mult)
                nc.vector.scalar_tensor_tensor(
                    out=Tt, in0=dT, scalar=step, op0=A.subtract, op1=A.add,
                    in1=Tt)
            return Tt

        med = bisect(r16, "med")
        negmed = pp.tile([128, 1], f32, name="negmed", tag="negmed")
        nc.vector.tensor_scalar(out=negmed, in0=med, scalar1=-1.0,
                                scalar2=None, op0=A.mult)
        u16 = pp.tile([128, 512], f16, name="u16", tag="u16")
        nc.scalar.activation(out=u16, in_=r16, func=AF.Abs,
                             bias=negmed[:, 0:1], scale=1.0)
        mad = bisect(u16, "mad")

        # --- loss = 0.5 * sum(w * d), w = relu(1 - d/(TUNE*std)^2)^2 ------
        c1 = pp.tile([128, 1], f32, name="c1", tag="c1")
        nc.vector.tensor_scalar(out=c1, in0=mad, scalar1=TUNE / MADSTD,
                                scalar2=None, op0=A.mult)
        cs2 = pp.tile([128, 1], f32, name="cs2", tag="cs2")
        nc.vector.tensor_tensor(out=cs2, in0=c1, in1=c1, op=A.mult)
        inv = pp.tile([128, 1], f32, name="inv", tag="inv")
        nc.vector.reciprocal(inv, cs2)

        t1 = pp.tile([128, 512], f32, name="t1", tag="t1")
        nc.vector.tensor_scalar(out=t1, in0=g, scalar1=inv[:, 0:1],
                                scalar2=None, op0=A.mult)
        v = pp.tile([128, 512], f32, name="v", tag="v")
        nc.scalar.activation(out=v, in_=t1, func=AF.Relu,
                             bias=1.0, scale=-1.0)
        y = pp.tile([128, 512], f32, name="y", tag="y")
        nc.vector.tensor_tensor(out=y, in0=v, in1=g, op=A.mult)
        S = pp.tile([128, 1], f32, name="S", tag="S")
        jkf = pp.tile([128, 512], f32, name="jkf", tag="jkf")
        nc.vector.scalar_tensor_tensor(
            out=jkf, in0=y, scalar=1.0, op0=A.bypass, op1=A.mult,
            in1=v, accum_out=S)

        ls = bp.tile([1, 1], f32, name="ls")
        nc.tensor.matmul(ls, lhsT=half1, rhs=S, start=True, stop=True)
        ls_sb = pp.tile([1, 1], f32, name="ls_sb", tag="ls_sb")
        nc.scalar.copy(out=ls_sb, in_=ls)
        nc.sync.dma_start(out=out_d, in_=ls_sb)

    from contextlib import ExitStack
    with tile.TileContext(nc) as tc, ExitStack() as stack:
        pp = stack.enter_context(tc.tile_pool(name="persist", bufs=1))
        emit(tc, pp, stack)

    nc.compile()
    return nc


def _split16(x64, dt):
    hi = x64.astype(dt)
    lo = (x64 - hi.astype(np.float64)).astype(dt)
    return hi, lo


def _shard_inputs(points3d_obs, points3d_pred):
    import ml_dtypes
    bf16 = ml_dtypes.bfloat16
    obs = np.asarray(points3d_obs, dtype=np.float32).reshape(BT, N_OBS, 3)
    pred = np.asarray(points3d_pred, dtype=np.float32).reshape(BT, M_PRED, 3)
    in_maps = []
    for core in range(NCORES):
        so = obs[core * F:(core + 1) * F]       # [F, N, 3]
        sp = pred[core * F:(core + 1) * F]      # [F, M, 3]

        ha, la = _split16(so.astype(np.float64), bf16)
        hna, lna = _split16(-0.5 * (so.astype(np.float64) ** 2).sum(-1), bf16)
        hb, lb = _split16(sp.astype(np.float64), bf16)
        hnb, lnb = _split16(-0.5 * (sp.astype(np.float64) ** 2).sum(-1), bf16)

        onesN = np.ones((F, N_OBS), bf16)
        onesM = np.ones((F, M_PRED), bf16)

        # [13, F*N]: hi/lo(-0.5|a|^2), ha, la, ha, 1, 1
        obs_rows = np.stack([
            hna, lna,
            ha[..., 0], ha[..., 1], ha[..., 2],
            la[..., 0], la[..., 1], la[..., 2],
            ha[..., 0], ha[..., 1], ha[..., 2],
            onesN, onesN,
        ], axis=0).reshape(13, F * N_OBS)
        # [13, F*M]: 1, 1, hb, hb, lb, hi/lo(-0.5|b|^2)
        pred_rows = np.stack([
            onesM, onesM,
            hb[..., 0], hb[..., 1], hb[..., 2],
            hb[..., 0], hb[..., 1], hb[..., 2],
            lb[..., 0], lb[..., 1], lb[..., 2],
            hnb, lnb,
        ], axis=0).reshape(13, F * M_PRED)

        in_maps.append({
            "obs_in": np.ascontiguousarray(obs_rows),
            "pred_in": np.ascontiguousarray(pred_rows),
        })
    return in_maps


def _get_nc(stage="D"):
    key = f"nc_{stage}"
    if key not in _CACHE:
        _CACHE[key] = _build_nc(stage)
    return _CACHE[key]


def run(points3d_obs, points3d_pred, stage="D", **kwargs):
    """Run on hardware; kwargs forwarded to run_bass_kernel_spmd."""
    from concourse.bass_utils import run_bass_kernel_spmd
    nc = _get_nc(stage)
    in_maps = _shard_inputs(points3d_obs, points3d_pred)
    res = run_bass_kernel_spmd(nc, in_maps, list(range(NCORES)), **kwargs)
    return res


def kernel(points3d_obs, points3d_pred):
    res = run(points3d_obs, points3d_pred)
    loss = (np.float32(res.results[0]["out"][0, 0])
            + np.float32(res.results[4]["out"][0, 0]))
    return np.asarray(loss, dtype=np.float32).reshape(())
